# revision 8
# baseline (speedup 1.0000x reference)
"""Trainium2 Bass kernel for nn_MetaUpSample (2x meta-upsample, 3x3 dynamic filters).

out[b,ho,wo,f] = sum_k patches[b,ho,wo,k] * meta_w[b,ho,wo,k*3+f]
  patches[b,ho,wo,(dk0,dk1,c)] = x_pad[b, ho//2+dk0, wo//2+dk1, c]

Sharding: 8 cores, core ci handles b = ci//2, ho in [(ci%2)*64, (ci%2)*64+64).
Both streams are bf16 on device (rel err ~2.5e-3 vs the 2e-2 gate), which
halves HBM traffic to ~30 MiB/core; the DMA stream runs ~307 GB/s -> ~98us.

The limiting on-chip resource (HW-calibrated) is SBUF array bandwidth
(~3.3 32-bit words/ns/core shared by all engines + DMA), so the compute
split minimizes SBUF word traffic while keeping each engine under ~96us:
  - path A (DVE scalar_tensor_tensor, 679 ns/unit): fused mult+reduce,
    mandatory dump written to PSUM -> 1152 SBUF words per 576-elem unit.
  - path C (DVE tensor_tensor bf16 2x, 369 ns + ACT activation-reduce,
    852 ns): TT writes packed bf16 products to an SBUF ring (864 words),
    ACT reads them (576) and dumps to PSUM; accumulates in fp32.
Defaults: 81 A-units, 111 C-units per core per 192 (row,filter) units.
(GpSimd TT was measured 1238ns/unit but costs 2880 SBUF words/unit and
stretched every other engine ~1.8x - total SBUF words matter more than
spare engine cycles. TensorTensorReduce / tensor_scalar+accum / STT-on-Pool
do not compile; DVE STT has no bf16 fast path on NC_v3; plain TT does.)

Host side pre-builds (a) the duplicated patch-row tensor xrb (x is only 4 MiB:
xrb[wo, hp, :] = the 192 bf16 of padded x row hp that pixel column wo needs),
(b) a fully linearized per-core meta_w layout [WO, HO_PC*F*K] so the device
meta_w DMA reads one contiguous 13.5KiB chunk per partition per 4-row tile,
and un-transposes the [wo, (ho,f)] device output.
"""
from contextlib import ExitStack

import numpy as np
import ml_dtypes

import concourse.bass as bass
import concourse.mybir as mybir
from concourse.bass_utils import run_bass_kernel_spmd

B, H, W, C = 4, 64, 64, 64
HO, WO, F = 128, 128, 3
KS = 3
K = KS * KS * C            # 576
QF = K * F                 # 1728 meta_w channels
RW = KS * C                # 192 bf16 per patch row (dk1, c)
N_CORES = 8
CORES_PER_B = N_CORES // B         # 2
HO_PC = HO // CORES_PER_B          # 64 output rows per core
NHS = HO_PC // 2                   # 32 hs tiles per core
NROWS = NHS + 2                    # 34 cached padded x rows per core

import os

NBUF = int(os.environ.get("K_NBUF", "3"))    # meta_w double-buffer slots
RPT = int(os.environ.get("K_RPT", "4"))      # meta_w rows per DMA tile
NSCRC = int(os.environ.get("K_NSCRC", "6"))  # TT->ACT product ring slots
YACT = int(os.environ.get("K_YACT", "111"))  # C-path units (ACT reduces)
PSUMV = os.environ.get("K_PSUMV", "1") == "1"  # STT dump -> PSUM
PSUMA = os.environ.get("K_PSUMA", "1") == "1"  # ACT dump -> PSUM
RING2 = os.environ.get("K_RING2", "0") == "1"  # odd meta_w tiles on ACT ring
HEAD = os.environ.get("K_HEAD", "1") == "1"    # small ramp-in tiles
FUSE = os.environ.get("K_FUSE", "1") == "1"    # fuse 2-C rows into one TT
XH = int(os.environ.get("K_XH", "9"))  # xrb head rows in the first chunk
XREST_AFTER = int(os.environ.get("K_XREST", "1"))  # rows computed before xrb-rest issues
XSYNC = os.environ.get("K_XSYNC", "1") == "1"  # xrb head on sync ring after tile 0

f32 = mybir.dt.float32
bf16 = mybir.dt.bfloat16
BF = ml_dtypes.bfloat16

TAIL = os.environ.get("K_TAIL", "1") == "1"
# small HEAD tiles let DVE start ~6us earlier (tile 0 = 1 row instead of 4);
# small TAIL tiles leave only ~1 row of compute after the last meta_w byte.
SCHED = [RPT] * (HO_PC // RPT)
if HEAD:
    SCHED = [1, 1, 2] + SCHED[1:]
if TAIL:
    SCHED = SCHED[:-1] + [2, 1, 1]
assert sum(SCHED) == HO_PC
NT = len(SCHED)
ROW0 = [sum(SCHED[:t]) for t in range(NT)]          # first ho row of tile t

# Per-(row, filter) path assignment. f2 (and f1 on YACT-64 evenly-spread
# rows) -> path C; the rest -> path A. Per row, C-units are emitted first
# on DVE so ACT's operand is ready as early as possible.
NC1 = max(0, min(HO_PC, YACT - HO_PC))
_c1_rows = set(np.linspace(0, HO_PC - 1, NC1, dtype=int).tolist()) if NC1 else set()
C_F = [([1] if r in _c1_rows else []) + [2] for r in range(HO_PC)]
A_F = [[0] + ([] if r in _c1_rows else [1]) for r in range(HO_PC)]
C_PRE = np.cumsum([0] + [len(C_F[r]) for r in range(HO_PC)]).tolist()
# Every row has >=1 C-unit (YACT >= 64), emitted as ONE tensor_tensor op
# (1152-elem broadcast-fused when the row has two C filters and FUSE is on).
assert all(len(C_F[r]) >= 1 for r in range(HO_PC))
# DVE ops per row: 1 TT (or 2 unfused) + len(A_F) STTs; TT(s) first.
V_OPS = [
    ((1 if FUSE else len(C_F[r])) if C_F[r] else 0) + len(A_F[r])
    for r in range(HO_PC)
]
V_PRE = np.cumsum([0] + V_OPS).tolist()

_CACHED = None


def _build_nc():
    # Cross-engine ordering is fully explicit via semaphores below; the
    # remaining same-engine WAW (PSUM dumps, scratch rings) is safe on HW
    # because each engine drains its pipe between ops.
    nc = bass.Bass(detect_race_conditions=False)
    mw_d = nc.declare_dram_parameter("mw", [WO, HO_PC * QF], bf16, isOutput=False)
    xrb_d = nc.declare_dram_parameter("xrb", [WO, NROWS * RW], bf16, isOutput=False)
    out_d = nc.declare_dram_parameter("out", [WO, HO_PC * F], f32, isOutput=True)

    with ExitStack() as ctx:
        xrow = ctx.enter_context(nc.sbuf_tensor([WO, NROWS * RW], bf16))
        mwbuf = ctx.enter_context(nc.sbuf_tensor([WO, NBUF * RPT * QF], bf16))
        scr_c = ctx.enter_context(nc.sbuf_tensor([WO, NSCRC * 2 * K], bf16))
        out_sb = ctx.enter_context(nc.sbuf_tensor([WO, HO_PC * F], f32))
        if PSUMV:
            dump_v = ctx.enter_context(nc.psum_tensor([WO, K], f32))
        else:
            dump_v = ctx.enter_context(nc.sbuf_tensor([WO, 2 * K], bf16))
        if PSUMA:
            dump_a = ctx.enter_context(nc.psum_tensor([WO, K], f32))
        else:
            dump_a = ctx.enter_context(nc.sbuf_tensor([WO, 2 * K], bf16))
        slot_sem = [ctx.enter_context(nc.semaphore(f"slot{j}")) for j in range(NBUF)]
        misc_sem = ctx.enter_context(nc.semaphore("misc"))
        cmp_v = ctx.enter_context(nc.semaphore("cmp_v"))   # DVE op count (3/row)
        cmp_a = ctx.enter_context(nc.semaphore("cmp_a"))   # ACT reduce count
        block = ctx.enter_context(nc.Block())

        def slot_ap(j, rows):
            base = j * RPT * QF
            return mwbuf[:, base : base + rows * QF]

        def mw_tile_dma(eng, i):
            j = i % NBUF
            rows, row0 = SCHED[i], ROW0[i]
            if i >= NBUF:
                prev = i - NBUF
                pe = ROW0[prev] + SCHED[prev]
                eng.wait_ge(cmp_v, V_PRE[pe])
            eng.dma_start(
                out=slot_ap(j, rows),
                in_=mw_d[:, row0 * QF : (row0 + rows) * QF],
            ).then_inc(slot_sem[j], 16)

        @block.sync
        def _(sync):
            for i in range(NT):
                if RING2 and i % 2 == 1:
                    continue
                mw_tile_dma(sync, i)
                if XSYNC and i == 0:
                    # xrb head right behind tile 0 on the same ring: costs
                    # tile 1 ~1.4us but keeps the boot window clean
                    sync.dma_start(
                        out=xrow[:, : XH * RW], in_=xrb_d[:, : XH * RW]
                    ).then_inc(misc_sem, 16)
            # overlap the bulk of the output store with the tail tiles
            sync.wait_ge(cmp_v, V_PRE[60])
            sync.wait_ge(cmp_a, C_PRE[60])
            sync.dma_start(
                out=out_d[:, : 60 * F], in_=out_sb[:, : 60 * F]
            ).then_inc(misc_sem, 16)
            sync.wait_ge(cmp_v, V_PRE[HO_PC])
            sync.wait_ge(cmp_a, C_PRE[HO_PC])
            sync.dma_start(
                out=out_d[:, 60 * F :], in_=out_sb[:, 60 * F :]
            ).then_inc(misc_sem, 16)

        @block.vector
        def _(vector):
            vector.wait_ge(misc_sem, 16)
            nv = 0   # DVE ops emitted
            nc_u = 0  # C-units emitted
            xrow_full_waited = False
            for i in range(NT):
                j, p = i % NBUF, i // NBUF
                rows = SCHED[i]
                if not xrow_full_waited and (ROW0[i] + rows - 1) // 2 + 2 >= XH:
                    vector.wait_ge(misc_sem, 32)  # rest of xrow loaded
                    xrow_full_waited = True
                vector.wait_ge(slot_sem[j], 16 * (p + 1))
                mw4 = slot_ap(j, rows).rearrange(
                    "p (h f k) -> p h f k", h=rows, f=F
                )
                for r in range(rows):
                    ho = ROW0[i] + r
                    win = xrow[:, (ho // 2) * RW : (ho // 2) * RW + KS * RW]
                    cs = C_F[ho]
                    slot = scr_c[:, (ho % NSCRC) * 2 * K : (ho % NSCRC) * 2 * K + len(cs) * K]
                    if ho >= NSCRC:
                        vector.wait_ge(cmp_a, C_PRE[ho - NSCRC + 1])
                    if FUSE or len(cs) == 1:
                        f0 = cs[0]
                        mws = mw4[:, r, f0 : f0 + len(cs), :]
                        if len(cs) > 1:
                            win_b, _ = bass.broadcast_tensor_aps(
                                win.rearrange("p (one k) -> p one k", one=1),
                                mws,
                            )
                            slot3 = slot.rearrange("p (c k) -> p c k", c=len(cs))
                            vector.tensor_tensor(
                                out=slot3, in0=mws, in1=win_b,
                                op=mybir.AluOpType.mult,
                            ).then_inc(cmp_v, 1)
                        else:
                            vector.tensor_tensor(
                                out=slot, in0=mw4[:, r, f0, :], in1=win,
                                op=mybir.AluOpType.mult,
                            ).then_inc(cmp_v, 1)
                        nv += 1
                    else:
                        for ci_, f in enumerate(cs):
                            vector.tensor_tensor(
                                out=slot[:, ci_ * K : (ci_ + 1) * K],
                                in0=mw4[:, r, f, :], in1=win,
                                op=mybir.AluOpType.mult,
                            ).then_inc(cmp_v, 1)
                            nv += 1
                    for f in A_F[ho]:
                        vector.scalar_tensor_tensor(
                            out=dump_v[:, :K] if PSUMV
                            else dump_v[:, (nv % 2) * K : (nv % 2 + 1) * K],
                            in0=mw4[:, r, f, :],
                            scalar=1.0,
                            in1=win,
                            op0=mybir.AluOpType.mult,
                            op1=mybir.AluOpType.mult,
                            accum_out=out_sb[:, ho * F + f : ho * F + f + 1],
                        ).then_inc(cmp_v, 1)
                        nv += 1

        @block.scalar
        def _(scalar):
            # xrb on the ACT HWDGE ring. The 16 DMA engines are SHARED
            # across rings, so the bulky xrb-rest chunk must not preempt the
            # first meta_w tiles: only the head goes out at t=0; the rest is
            # held back until the first row of compute confirms the stream
            # is rolling (it still lands long before row 2*XH-5 needs it).
            if not XSYNC:
                scalar.dma_start(
                    out=xrow[:, : XH * RW], in_=xrb_d[:, : XH * RW]
                ).then_inc(misc_sem, 16)
            scalar.wait_ge(cmp_v, V_PRE[XREST_AFTER])
            scalar.dma_start(
                out=xrow[:, XH * RW :], in_=xrb_d[:, XH * RW :]
            ).then_inc(misc_sem, 16)
            if RING2:
                pending = [i for i in range(NT) if i % 2 == 1]
            else:
                pending = []
            na = 0
            for ho in range(HO_PC):
                while pending and (
                    pending[0] < NBUF
                    or ROW0[pending[0] - NBUF] + SCHED[pending[0] - NBUF] <= ho
                ):
                    mw_tile_dma(scalar, pending.pop(0))
                cs = C_F[ho]
                n_tt = 1 if (FUSE or len(cs) == 1) else len(cs)
                for ci_, f in enumerate(cs):
                    w_tt = n_tt if FUSE else ci_ + 1
                    scalar.wait_ge(cmp_v, V_PRE[ho] + w_tt)
                    scalar.activation(
                        out=dump_a[:, :K] if PSUMA
                        else dump_a[:, (na % 2) * K : (na % 2 + 1) * K],
                        in_=scr_c[:, (ho % NSCRC) * 2 * K + ci_ * K
                                  : (ho % NSCRC) * 2 * K + (ci_ + 1) * K],
                        func=mybir.ActivationFunctionType.Copy,
                        accum_out=out_sb[:, ho * F + f : ho * F + f + 1],
                    ).then_inc(cmp_a, 1)
                    na += 1
            for i in pending:
                mw_tile_dma(scalar, i)

    return nc


def _prep_xrb(x):
    """Per-core duplicated patch-row tensors (bf16).

    xrb[ci][wo, hpl*RW + dk1*C + c] = x_pad[b, hs0+hpl, wo//2 + dk1, c]
    where x_pad has 1 zero row/col of padding on each side.
    """
    from numpy.lib.stride_tricks import sliding_window_view

    out = []
    for ci in range(N_CORES):
        b, hs0 = ci // CORES_PER_B, (ci % CORES_PER_B) * NHS
        xp = np.pad(x[b], ((1, 1), (1, 1), (0, 0)))          # [66, 66, 64]
        rows = xp[hs0 : hs0 + NROWS]                          # [34, 66, 64]
        win = sliding_window_view(rows, KS, axis=1)           # [34, 64(ws), 64(c), 3(dk1)]
        win = win.transpose(0, 1, 3, 2).reshape(NROWS, W, RW)  # [34, 64, 192]
        dup = np.repeat(win, 2, axis=1)                       # [34, 128, 192]
        out.append(
            np.ascontiguousarray(
                dup.transpose(1, 0, 2).astype(BF)
            ).reshape(WO, NROWS * RW)
        )
    return out


def _ensure_axon_hooks_module():
    """This image's antenv lacks axon_hooks; run_bass_kernel_spmd imports it
    when BASS_TRACE is set. Provide it (registering the real NTFF hook when
    available) so tracing degrades gracefully instead of crashing."""
    try:
        import antenv.axon_hooks  # noqa: F401
        return
    except ImportError:
        pass
    import sys
    import types

    try:
        import antenv
    except ImportError:
        return
    mod = types.ModuleType("antenv.axon_hooks")
    _hook = [None]
    mod.set_axon_ntff_profile_hook = lambda h: _hook.__setitem__(0, h)
    mod.get_axon_ntff_profile_hook = lambda: _hook[0]
    sys.modules["antenv.axon_hooks"] = mod
    antenv.axon_hooks = mod
    try:
        from trn_agent_boot.trn_boot import _ntff_profile_via_ctypes

        h = _ntff_profile_via_ctypes("/opt/axon/libaxon_pjrt.so")
        if h is not None:
            _hook[0] = h
    except Exception:
        pass


_ensure_axon_hooks_module()

last_results = None  # BassKernelResults of the most recent kernel() call


def kernel(x, meta_w):
    global _CACHED, last_results
    x = np.ascontiguousarray(np.asarray(x, dtype=np.float32))
    meta_w = np.asarray(meta_w, dtype=np.float32)

    if _CACHED is None:
        _CACHED = _build_nc()
    nc = _CACHED

    xrbs = _prep_xrb(x)
    mw16 = meta_w.astype(BF)  # one pass over the full 432 MiB, then 2B moves
    in_maps = []
    for ci in range(N_CORES):
        b, ho0 = ci // CORES_PER_B, (ci % CORES_PER_B) * HO_PC
        mw_c = mw16[b, ho0 : ho0 + HO_PC].reshape(HO_PC, WO, K, F)
        # [WO, HO_PC, F, K]: device DMA reads are contiguous per partition,
        # and the per-(ho,f) operand slice of K is contiguous in SBUF.
        mw_c = np.ascontiguousarray(mw_c.transpose(1, 0, 3, 2)).reshape(
            WO, HO_PC * QF
        )
        in_maps.append({"mw": mw_c, "xrb": xrbs[ci]})

    res = run_bass_kernel_spmd(nc, in_maps, list(range(N_CORES)))
    last_results = res

    out = np.empty((B, HO, WO, F), np.float32)
    for ci in range(N_CORES):
        b, ho0 = ci // CORES_PER_B, (ci % CORES_PER_B) * HO_PC
        o = res.results[ci]["out"].reshape(WO, HO_PC, F)
        out[b, ho0 : ho0 + HO_PC] = o.transpose(1, 0, 2)
    return out


# revision 9
# speedup vs baseline: 1.2002x; 1.2002x over previous
"""Trainium2 Bass kernel for nn_MetaUpSample (2x meta-upsample, 3x3 dynamic filters).

out[b,ho,wo,f] = sum_k patches[b,ho,wo,k] * meta_w[b,ho,wo,k*3+f]
  patches[b,ho,wo,(dk0,dk1,c)] = x_pad[b, ho//2+dk0, wo//2+dk1, c]

Sharding: 8 cores, core ci handles b = ci//2, ho in [(ci%2)*64, (ci%2)*64+64).
Both streams are bf16 on device (rel err ~2.5e-3 vs the 2e-2 gate), which
halves HBM traffic to ~30 MiB/core; the DMA stream runs ~307 GB/s -> ~98us.

The limiting on-chip resource (HW-calibrated) is SBUF array bandwidth
(~3.3 32-bit words/ns/core shared by all engines + DMA), so the compute
split minimizes SBUF word traffic while keeping each engine under ~96us:
  - path A (DVE scalar_tensor_tensor, 679 ns/unit): fused mult+reduce,
    mandatory dump written to PSUM -> 1152 SBUF words per 576-elem unit.
  - path C (DVE tensor_tensor bf16 2x, 369 ns + ACT activation-reduce,
    852 ns): TT writes packed bf16 products to an SBUF ring (864 words),
    ACT reads them (576) and dumps to PSUM; accumulates in fp32.
Defaults: 81 A-units, 111 C-units per core per 192 (row,filter) units.
(GpSimd TT was measured 1238ns/unit but costs 2880 SBUF words/unit and
stretched every other engine ~1.8x - total SBUF words matter more than
spare engine cycles. TensorTensorReduce / tensor_scalar+accum / STT-on-Pool
do not compile; DVE STT has no bf16 fast path on NC_v3; plain TT does.)

Host side pre-builds (a) the duplicated patch-row tensor xrb (x is only 4 MiB:
xrb[wo, hp, :] = the 192 bf16 of padded x row hp that pixel column wo needs),
(b) a fully linearized per-core meta_w layout [WO, HO_PC*F*K] so the device
meta_w DMA reads one contiguous 13.5KiB chunk per partition per 4-row tile,
and un-transposes the [wo, (ho,f)] device output.
"""
from contextlib import ExitStack

import numpy as np
import ml_dtypes

import concourse.bass as bass
import concourse.mybir as mybir
from concourse.bass_utils import run_bass_kernel_spmd

B, H, W, C = 4, 64, 64, 64
HO, WO, F = 128, 128, 3
KS = 3
K = KS * KS * C            # 576
QF = K * F                 # 1728 meta_w channels
RW = KS * C                # 192 bf16 per patch row (dk1, c)
N_CORES = 8
CORES_PER_B = N_CORES // B         # 2
HO_PC = HO // CORES_PER_B          # 64 output rows per core
NHS = HO_PC // 2                   # 32 hs tiles per core
NROWS = NHS + 2                    # 34 cached padded x rows per core

import os

NBUF = int(os.environ.get("K_NBUF", "5"))    # meta_w double-buffer slots
RPT = int(os.environ.get("K_RPT", "4"))      # meta_w rows per DMA tile
NSCRC = int(os.environ.get("K_NSCRC", "6"))  # TT->ACT product ring slots
YACT = int(os.environ.get("K_YACT", "111"))  # C-path units (ACT reduces)
PSUMV = os.environ.get("K_PSUMV", "1") == "1"  # STT dump -> PSUM
PSUMA = os.environ.get("K_PSUMA", "1") == "1"  # ACT dump -> PSUM
RING2 = os.environ.get("K_RING2", "0") == "1"  # odd meta_w tiles on ACT ring
HEAD = os.environ.get("K_HEAD", "1") == "1"    # small ramp-in tiles
FUSE = os.environ.get("K_FUSE", "1") == "1"    # fuse 2-C rows into one TT
XH = int(os.environ.get("K_XH", "9"))  # xrb head rows in the first chunk
XREST_AFTER = int(os.environ.get("K_XREST", "1"))  # rows computed before xrb-rest issues
XSYNC = os.environ.get("K_XSYNC", "0") == "1"  # xrb head on sync ring after tile 0

f32 = mybir.dt.float32
bf16 = mybir.dt.bfloat16
BF = ml_dtypes.bfloat16

TAIL = os.environ.get("K_TAIL", "1") == "1"
# small HEAD tiles let DVE start ~6us earlier (tile 0 = 1 row instead of 4);
# small TAIL tiles leave only ~1 row of compute after the last meta_w byte.
SCHED = [RPT] * (HO_PC // RPT)
if HEAD:
    SCHED = [1, 1, 2] + SCHED[1:]
if TAIL:
    SCHED = SCHED[:-1] + [2, 1, 1]
assert sum(SCHED) == HO_PC
NT = len(SCHED)
ROW0 = [sum(SCHED[:t]) for t in range(NT)]          # first ho row of tile t

# Per-(row, filter) path assignment. f2 (and f1 on YACT-64 evenly-spread
# rows) -> path C; the rest -> path A. Per row, C-units are emitted first
# on DVE so ACT's operand is ready as early as possible.
NC1 = max(0, min(HO_PC, YACT - HO_PC))
_c1_rows = set(np.linspace(0, HO_PC - 1, NC1, dtype=int).tolist()) if NC1 else set()
C_F = [([1] if r in _c1_rows else []) + [2] for r in range(HO_PC)]
A_F = [[0] + ([] if r in _c1_rows else [1]) for r in range(HO_PC)]
C_PRE = np.cumsum([0] + [len(C_F[r]) for r in range(HO_PC)]).tolist()
# Every row has >=1 C-unit (YACT >= 64), emitted as ONE tensor_tensor op
# (1152-elem broadcast-fused when the row has two C filters and FUSE is on).
assert all(len(C_F[r]) >= 1 for r in range(HO_PC))
# DVE ops per row: 1 TT (or 2 unfused) + len(A_F) STTs; TT(s) first.
V_OPS = [
    ((1 if FUSE else len(C_F[r])) if C_F[r] else 0) + len(A_F[r])
    for r in range(HO_PC)
]
V_PRE = np.cumsum([0] + V_OPS).tolist()

_CACHED = None


def _build_nc():
    # Cross-engine ordering is fully explicit via semaphores below; the
    # remaining same-engine WAW (PSUM dumps, scratch rings) is safe on HW
    # because each engine drains its pipe between ops.
    nc = bass.Bass(detect_race_conditions=False)
    mw_d = nc.declare_dram_parameter("mw", [WO, HO_PC * QF], bf16, isOutput=False)
    xrb_d = nc.declare_dram_parameter("xrb", [WO, NROWS * RW], bf16, isOutput=False)
    out_d = nc.declare_dram_parameter("out", [WO, HO_PC * F], f32, isOutput=True)

    with ExitStack() as ctx:
        xrow = ctx.enter_context(nc.sbuf_tensor([WO, NROWS * RW], bf16))
        mwbuf = ctx.enter_context(nc.sbuf_tensor([WO, NBUF * RPT * QF], bf16))
        scr_c = ctx.enter_context(nc.sbuf_tensor([WO, NSCRC * 2 * K], bf16))
        out_sb = ctx.enter_context(nc.sbuf_tensor([WO, HO_PC * F], f32))
        if PSUMV:
            dump_v = ctx.enter_context(nc.psum_tensor([WO, K], f32))
        else:
            dump_v = ctx.enter_context(nc.sbuf_tensor([WO, 2 * K], bf16))
        if PSUMA:
            dump_a = ctx.enter_context(nc.psum_tensor([WO, K], f32))
        else:
            dump_a = ctx.enter_context(nc.sbuf_tensor([WO, 2 * K], bf16))
        slot_sem = [ctx.enter_context(nc.semaphore(f"slot{j}")) for j in range(NBUF)]
        misc_sem = ctx.enter_context(nc.semaphore("misc"))
        cmp_v = ctx.enter_context(nc.semaphore("cmp_v"))   # DVE op count (3/row)
        cmp_a = ctx.enter_context(nc.semaphore("cmp_a"))   # ACT reduce count
        block = ctx.enter_context(nc.Block())

        def slot_ap(j, rows):
            base = j * RPT * QF
            return mwbuf[:, base : base + rows * QF]

        def mw_tile_dma(eng, i):
            j = i % NBUF
            rows, row0 = SCHED[i], ROW0[i]
            if i >= NBUF:
                prev = i - NBUF
                pe = ROW0[prev] + SCHED[prev]
                eng.wait_ge(cmp_v, V_PRE[pe])
            eng.dma_start(
                out=slot_ap(j, rows),
                in_=mw_d[:, row0 * QF : (row0 + rows) * QF],
            ).then_inc(slot_sem[j], 16)

        @block.sync
        def _(sync):
            for i in range(NT):
                if RING2 and i % 2 == 1:
                    continue
                mw_tile_dma(sync, i)
                if XSYNC and i == 0:
                    # xrb head right behind tile 0 on the same ring: costs
                    # tile 1 ~1.4us but keeps the boot window clean
                    sync.dma_start(
                        out=xrow[:, : XH * RW], in_=xrb_d[:, : XH * RW]
                    ).then_inc(misc_sem, 16)
            # overlap the bulk of the output store with the tail tiles
            sync.wait_ge(cmp_v, V_PRE[60])
            sync.wait_ge(cmp_a, C_PRE[60])
            sync.dma_start(
                out=out_d[:, : 60 * F], in_=out_sb[:, : 60 * F]
            ).then_inc(misc_sem, 16)
            sync.wait_ge(cmp_v, V_PRE[HO_PC])
            sync.wait_ge(cmp_a, C_PRE[HO_PC])
            sync.dma_start(
                out=out_d[:, 60 * F :], in_=out_sb[:, 60 * F :]
            ).then_inc(misc_sem, 16)

        @block.vector
        def _(vector):
            vector.wait_ge(misc_sem, 16)
            nv = 0   # DVE ops emitted
            nc_u = 0  # C-units emitted
            xrow_full_waited = False
            for i in range(NT):
                j, p = i % NBUF, i // NBUF
                rows = SCHED[i]
                if not xrow_full_waited and (ROW0[i] + rows - 1) // 2 + 2 >= XH:
                    vector.wait_ge(misc_sem, 32)  # rest of xrow loaded
                    xrow_full_waited = True
                vector.wait_ge(slot_sem[j], 16 * (p + 1))
                mw4 = slot_ap(j, rows).rearrange(
                    "p (h f k) -> p h f k", h=rows, f=F
                )
                for r in range(rows):
                    ho = ROW0[i] + r
                    win = xrow[:, (ho // 2) * RW : (ho // 2) * RW + KS * RW]
                    cs = C_F[ho]
                    slot = scr_c[:, (ho % NSCRC) * 2 * K : (ho % NSCRC) * 2 * K + len(cs) * K]
                    if ho >= NSCRC:
                        vector.wait_ge(cmp_a, C_PRE[ho - NSCRC + 1])
                    if FUSE or len(cs) == 1:
                        f0 = cs[0]
                        mws = mw4[:, r, f0 : f0 + len(cs), :]
                        if len(cs) > 1:
                            win_b, _ = bass.broadcast_tensor_aps(
                                win.rearrange("p (one k) -> p one k", one=1),
                                mws,
                            )
                            slot3 = slot.rearrange("p (c k) -> p c k", c=len(cs))
                            vector.tensor_tensor(
                                out=slot3, in0=mws, in1=win_b,
                                op=mybir.AluOpType.mult,
                            ).then_inc(cmp_v, 1)
                        else:
                            vector.tensor_tensor(
                                out=slot, in0=mw4[:, r, f0, :], in1=win,
                                op=mybir.AluOpType.mult,
                            ).then_inc(cmp_v, 1)
                        nv += 1
                    else:
                        for ci_, f in enumerate(cs):
                            vector.tensor_tensor(
                                out=slot[:, ci_ * K : (ci_ + 1) * K],
                                in0=mw4[:, r, f, :], in1=win,
                                op=mybir.AluOpType.mult,
                            ).then_inc(cmp_v, 1)
                            nv += 1
                    for f in A_F[ho]:
                        vector.scalar_tensor_tensor(
                            out=dump_v[:, :K] if PSUMV
                            else dump_v[:, (nv % 2) * K : (nv % 2 + 1) * K],
                            in0=mw4[:, r, f, :],
                            scalar=1.0,
                            in1=win,
                            op0=mybir.AluOpType.mult,
                            op1=mybir.AluOpType.mult,
                            accum_out=out_sb[:, ho * F + f : ho * F + f + 1],
                        ).then_inc(cmp_v, 1)
                        nv += 1

        @block.scalar
        def _(scalar):
            # xrb on the ACT HWDGE ring. The 16 DMA engines are SHARED
            # across rings, so the bulky xrb-rest chunk must not preempt the
            # first meta_w tiles: only the head goes out at t=0; the rest is
            # held back until the first row of compute confirms the stream
            # is rolling (it still lands long before row 2*XH-5 needs it).
            if not XSYNC:
                scalar.dma_start(
                    out=xrow[:, : XH * RW], in_=xrb_d[:, : XH * RW]
                ).then_inc(misc_sem, 16)
            scalar.wait_ge(cmp_v, V_PRE[XREST_AFTER])
            scalar.dma_start(
                out=xrow[:, XH * RW :], in_=xrb_d[:, XH * RW :]
            ).then_inc(misc_sem, 16)
            if RING2:
                pending = [i for i in range(NT) if i % 2 == 1]
            else:
                pending = []
            na = 0
            for ho in range(HO_PC):
                while pending and (
                    pending[0] < NBUF
                    or ROW0[pending[0] - NBUF] + SCHED[pending[0] - NBUF] <= ho
                ):
                    mw_tile_dma(scalar, pending.pop(0))
                cs = C_F[ho]
                n_tt = 1 if (FUSE or len(cs) == 1) else len(cs)
                for ci_, f in enumerate(cs):
                    w_tt = n_tt if FUSE else ci_ + 1
                    scalar.wait_ge(cmp_v, V_PRE[ho] + w_tt)
                    scalar.activation(
                        out=dump_a[:, :K] if PSUMA
                        else dump_a[:, (na % 2) * K : (na % 2 + 1) * K],
                        in_=scr_c[:, (ho % NSCRC) * 2 * K + ci_ * K
                                  : (ho % NSCRC) * 2 * K + (ci_ + 1) * K],
                        func=mybir.ActivationFunctionType.Copy,
                        accum_out=out_sb[:, ho * F + f : ho * F + f + 1],
                    ).then_inc(cmp_a, 1)
                    na += 1
            for i in pending:
                mw_tile_dma(scalar, i)

    return nc


def _prep_xrb(x):
    """Per-core duplicated patch-row tensors (bf16).

    xrb[ci][wo, hpl*RW + dk1*C + c] = x_pad[b, hs0+hpl, wo//2 + dk1, c]
    where x_pad has 1 zero row/col of padding on each side.
    """
    from numpy.lib.stride_tricks import sliding_window_view

    out = []
    for ci in range(N_CORES):
        b, hs0 = ci // CORES_PER_B, (ci % CORES_PER_B) * NHS
        xp = np.pad(x[b], ((1, 1), (1, 1), (0, 0)))          # [66, 66, 64]
        rows = xp[hs0 : hs0 + NROWS]                          # [34, 66, 64]
        win = sliding_window_view(rows, KS, axis=1)           # [34, 64(ws), 64(c), 3(dk1)]
        win = win.transpose(0, 1, 3, 2).reshape(NROWS, W, RW)  # [34, 64, 192]
        dup = np.repeat(win, 2, axis=1)                       # [34, 128, 192]
        out.append(
            np.ascontiguousarray(
                dup.transpose(1, 0, 2).astype(BF)
            ).reshape(WO, NROWS * RW)
        )
    return out


def _ensure_axon_hooks_module():
    """This image's antenv lacks axon_hooks; run_bass_kernel_spmd imports it
    when BASS_TRACE is set. Provide it (registering the real NTFF hook when
    available) so tracing degrades gracefully instead of crashing."""
    try:
        import antenv.axon_hooks  # noqa: F401
        return
    except ImportError:
        pass
    import sys
    import types

    try:
        import antenv
    except ImportError:
        return
    mod = types.ModuleType("antenv.axon_hooks")
    _hook = [None]
    mod.set_axon_ntff_profile_hook = lambda h: _hook.__setitem__(0, h)
    mod.get_axon_ntff_profile_hook = lambda: _hook[0]
    sys.modules["antenv.axon_hooks"] = mod
    antenv.axon_hooks = mod
    try:
        from trn_agent_boot.trn_boot import _ntff_profile_via_ctypes

        h = _ntff_profile_via_ctypes("/opt/axon/libaxon_pjrt.so")
        if h is not None:
            _hook[0] = h
    except Exception:
        pass


_ensure_axon_hooks_module()

last_results = None  # BassKernelResults of the most recent kernel() call


def kernel(x, meta_w):
    global _CACHED, last_results
    x = np.ascontiguousarray(np.asarray(x, dtype=np.float32))
    meta_w = np.asarray(meta_w, dtype=np.float32)

    if _CACHED is None:
        _CACHED = _build_nc()
    nc = _CACHED

    xrbs = _prep_xrb(x)
    mw16 = meta_w.astype(BF)  # one pass over the full 432 MiB, then 2B moves
    in_maps = []
    for ci in range(N_CORES):
        b, ho0 = ci // CORES_PER_B, (ci % CORES_PER_B) * HO_PC
        mw_c = mw16[b, ho0 : ho0 + HO_PC].reshape(HO_PC, WO, K, F)
        # [WO, HO_PC, F, K]: device DMA reads are contiguous per partition,
        # and the per-(ho,f) operand slice of K is contiguous in SBUF.
        mw_c = np.ascontiguousarray(mw_c.transpose(1, 0, 3, 2)).reshape(
            WO, HO_PC * QF
        )
        in_maps.append({"mw": mw_c, "xrb": xrbs[ci]})

    res = run_bass_kernel_spmd(nc, in_maps, list(range(N_CORES)))
    last_results = res

    out = np.empty((B, HO, WO, F), np.float32)
    for ci in range(N_CORES):
        b, ho0 = ci // CORES_PER_B, (ci % CORES_PER_B) * HO_PC
        o = res.results[ci]["out"].reshape(WO, HO_PC, F)
        out[b, ho0 : ho0 + HO_PC] = o.transpose(1, 0, 2)
    return out
